# revision 43
# baseline (speedup 1.0000x reference)
"""Multi-head causal attention on 8 Trainium2 NeuronCores — v3.

Sharding: tensor-parallel over heads (2 heads/core) for QKV + attention;
per-q-tile AllToAlls convert to token-sharding (tokens interleaved at
64-token granularity) so the collectives pipeline behind attention
compute, and each q-tile's output projection runs as soon as its A2A
lands. The host gather is pure concatenation (interleaved slices).

v4 schedule (vs v2):
  - score PSUM at k-tile granularity: one [128,1024] two-bank tile per
    k-tile holding BOTH heads (halves), pool bufs=2, so the
    scores->exp->AV pipeline runs 2 k-tiles deep (v2's per-pair tiles
    exhausted the pool with one pair, serializing MM->exp->MM).
  - ONE exp per k-tile ([128,1024] spanning both heads) -- ACT per-op
    overhead was the attention cadence limiter, which left the PE idle
    enough that HAM throttled it to 1.2 GHz for most of the kernel.
  - receiver-side softmax normalization: the A2A payload carries raw
    (unnormalized) AV rows plus the two denominator rows (130x64 per
    dest); after ab_load the receiving core computes 1/denom via a
    bit-trick+Newton step on the otherwise-idle GpSimd engine,
    broadcasts it with a tiny K=2 PE outer product, and applies one DVE
    multiply. The per-chunk sender epilogue shrinks to two cast-copies,
    removing the ~3-5us serial reciprocal chain from every chunk
    boundary.
  - causal mask multiply as one [128,1024] DVE op (mask tile doubled).
  - per-head score matmuls (K=64) auto-row-tile to PE quadrants
    (0,0)/(64,0) and run concurrently when not dependency-stalled.
  - chunk order descending qt: (0,3),(1,3),(0,2),(1,2),... so the last
    attention chunk is the smallest; one A2A per qt carrying both
    batches (4 collectives instead of 8, each ~5-7us incl. fixed cost);
    outproj(qt) interleaves right behind, leaving only a2a(0)+outproj(0)
    on the tail.
  - QKV projection chunks run upfront (batch 0 first, attention(0,3)
    emitted between the halves); the Tile scheduler interleaves qkv/
    outproj matmuls into attention stalls to keep the PE warm.
  - V transposed in bf16 (1 cyc/row vs 2), epilogue broadcast PSUM
    borrowed from the score pool.

Per-core layout:
  - everything bf16 on the wire (x, Wqkv, Wo, A2A payloads); f32 PSUM.
  - qkvT = W^T x^T in [feature, token] layout; V additionally
    PE-transposed to [token, feature] with a ones-column appended so the
    softmax denominator falls out of the AV accumulation (row 64).
  - softmax: exp on ScalarE with 1/sqrt(D) folded into the activation
    scale; no max-subtraction (scores are O(6)); causal masking is a
    multiplicative 0/1 bf16 mask on diagonal tiles.
  - normalize: 1/denom = exp(-ln(denom)) on ScalarE; PE outer product
    broadcasts it across partitions; single DVE multiply emits the bf16
    A2A payload.
"""

import numpy as np
import ml_dtypes

import concourse.bass as bass
import concourse.mybir as mybir
import concourse.tile as tile
from concourse.bass_utils import run_bass_kernel_spmd
from concourse.masks import make_identity
from concourse.vector_clock import ScopedClock

F32 = mybir.dt.float32
BF16 = mybir.dt.bfloat16
I32 = mybir.dt.int32
AF = mybir.ActivationFunctionType

# f32 reciprocal bit-trick seed: r0 = bits(0x7EF311C3 - bits(x)), |err|<~4%;
# one Newton step r1 = r0*(2 - x*r0) brings it to ~0.2%.
RECIP_MAGIC = 0x7EF311C3


def _install_cache_nonce_hook():
    """The libneuronxla NEFF cache hashes the HLO but the BIR rides in
    backend_config (excluded from the hash); inject a hash of the BIR into
    mhlo.frontend_attributes which IS part of the model hash."""
    import hashlib
    import concourse.bass2jax as bass2jax
    from jax.interpreters import mlir

    if getattr(bass2jax, "_ant_cache_nonce_hooked", False):
        return
    bass2jax._ant_cache_nonce_hooked = True
    orig = bass2jax._accumulate_module_dve_attrs

    def patched(ctx, nc):
        orig(ctx, nc)
        op = ctx.module_context.module.operation
        cur = (
            op.attributes["mhlo.frontend_attributes"]
            if "mhlo.frontend_attributes" in op.attributes
            else None
        )
        existing = (
            {a.name: mlir.ir.StringAttr(a.attr).value for a in cur}
            if cur is not None
            else {}
        )
        import os

        existing["ant.cache_nonce"] = hashlib.sha256(
            nc.to_json_bytes()
            + os.environ.get("BASS_LDW_OPT", "0").encode()
        ).hexdigest()
        op.attributes["mhlo.frontend_attributes"] = mlir.ir.DictAttr.get(
            {k: mlir.ir.StringAttr.get(v) for k, v in existing.items()}
        )

    bass2jax._accumulate_module_dve_attrs = patched


_install_cache_nonce_hook()


def _install_ldw_opt_hook():
    """Experimental: flip walrus to --enable-ldw-opt=true (dedupes redundant
    LDWEIGHTS). Guarded by BASS_LDW_OPT=1; all stationaries here are bf16."""
    import concourse.bass_utils as bu

    if getattr(bu, "_ant_ldw_opt_hooked", False):
        return
    bu._ant_ldw_opt_hooked = True
    orig = bu.run_command

    def patched(argv, **kwargs):
        argv = [
            "--enable-ldw-opt=true" if a == "--enable-ldw-opt=false" else a
            for a in argv
        ]
        return orig(argv, **kwargs)

    bu.run_command = patched


import os as _os

if _os.environ.get("BASS_LDW_OPT") == "1":
    _install_ldw_opt_hook()


B, S, DM = 2, 2048, 1024
H, D = 16, 64
NCORES = 8
HP = H // NCORES          # heads per core
T = B * S                 # 4096 tokens
NCH = 8                   # token chunks of 512 (b*4 + qt)
KT_PER_S = S // 128       # 16 k-tiles per sequence
QT_PER_S = S // 512       # 4 q-tiles per sequence
SCALE = 1.0 / np.sqrt(D)

MAX_WAITS = 1  # walrus in this container rejects >1 sem-wait per instruction


def _split_waits(nc, limit=MAX_WAITS):
    """Post-pass: move excess sem-waits onto preceding same-engine nops."""
    n_id = 0
    for bb in nc.main_func.blocks:
        new = []
        for inst in bb.instructions:
            si = getattr(inst, "sync_info", None)
            if si is not None and len(si.on_wait) > limit:
                waits = list(si.on_wait)
                for i in range(0, len(waits) - limit, limit):
                    nop = mybir.InstNoOp(
                        name=f"wsplit-{n_id}", ins=[], outs=[], engine=inst.engine
                    )
                    n_id += 1
                    nop.sync_info = mybir.SyncInfo(
                        on_wait=waits[i : i + limit], on_update=[]
                    )
                    new.append(nop)
                kept = waits[len(waits) - limit :]
                inst.sync_info = mybir.SyncInfo(
                    on_wait=kept, on_update=list(si.on_update)
                )
            new.append(inst)
        bb.instructions = new


class _TileCtx(tile.TileContext):
    """Split the tail drain's multi-waits (this walrus build rejects >1-2
    sem-waits per instruction)."""

    def _drain_and_barrier(self, tick_clock, wait_clock):
        nc = self.nc
        drain_inst = nc.sync.drain()
        wait_clock.add_sem_waits(
            drain_inst.ins, ScopedClock({None: tick_clock.global_clock})
        )
        si = drain_inst.ins.sync_info
        if si is not None and len(si.on_wait) > 1:
            waits = list(si.on_wait)
            drain_inst.ins.sync_info = mybir.SyncInfo(
                on_wait=[waits[0]], on_update=list(si.on_update)
            )
            for w in waits[1:]:
                nop = nc.sync.nop(nofuse=True, hint="tail_drain_wait_split")
                nop.ins.sync_info = mybir.SyncInfo(on_wait=[w], on_update=[])

        nc.all_engine_barrier()
        assert self.sems is not None
        popped = nc._tile_sem_poison_stack.pop()
        assert popped is self._sem_poison
        nc.clear_and_free_semaphores(list(self.sems.allocated().values()))
        nc.all_engine_barrier()


def _nkt(qt, mode):
    """Number of k-tiles attended by q-tile qt (within one sequence)."""
    return 4 * (qt + 1) if mode == "causal" else KT_PER_S


def build(mode, n_mask_tiles):
    """Build the SPMD Bass program. mode: 'causal' | 'full' | 'general'."""
    nc = bass.Bass()

    xT = nc.dram_tensor("xT", [DM, T], BF16, kind="ExternalInput")
    wq = nc.dram_tensor("wq", [DM, 128], BF16, kind="ExternalInput")
    wk = nc.dram_tensor("wk", [DM, 128], BF16, kind="ExternalInput")
    wv = nc.dram_tensor("wv", [DM, 128], BF16, kind="ExternalInput")
    wo = nc.dram_tensor("wo", [DM, DM], BF16, kind="ExternalInput")
    sel_t = nc.dram_tensor("sel", [2, 128], BF16, kind="ExternalInput")
    rmag_t = nc.dram_tensor("rmagic", [2, 1024], I32, kind="ExternalInput")
    if n_mask_tiles:
        mt = nc.dram_tensor(
            "mt",
            [n_mask_tiles, 128, 1024 if mode == "causal" else 512],
            BF16,
            kind="ExternalInput",
        )
    # out rows: [qt][64 tokens of b0 | 64 tokens of b1]; this core's token
    # slice of q-tile qt is [512*qt + 64*core_id, +64) in each batch.
    out = nc.dram_tensor("out", [4 * 128, DM], F32, kind="ExternalOutput")

    with _TileCtx(nc) as tc:
        with (
            tc.tile_pool(name="const", bufs=1) as const,
            tc.tile_pool(
                name="xin", bufs=4 if n_mask_tiles <= 4 else 3
            ) as xin,
            tc.tile_pool(name="stage", bufs=3) as stage,
            tc.tile_pool(name="pp", bufs=8) as pp,
            tc.tile_pool(name="misc", bufs=4) as misc,
            tc.tile_pool(name="ps_s", bufs=2, space="PSUM") as ps_s,
            tc.tile_pool(name="ps_av", bufs=2, space="PSUM") as ps_av,
            tc.tile_pool(name="ps_misc", bufs=2, space="PSUM") as ps_misc,
            tc.tile_pool(name="dram", bufs=1, space="DRAM") as dram,
        ):
            import contextlib

            _stk = contextlib.ExitStack()
            dramp = [
                _stk.enter_context(
                    tc.tile_pool(name=f"dram{qt}", bufs=1, space="DRAM")
                )
                for qt in range(QT_PER_S)
            ]
            # ---- resident SBUF tensors ----
            wq_sb = const.tile([128, 8, 128], BF16)
            wk_sb = const.tile([128, 8, 128], BF16)
            wv_sb = const.tile([128, 8, 128], BF16)
            nc.sync.dma_start(wq_sb[:], wq.rearrange("(o p) e -> p o e", p=128))
            xts = {}

            def load_x(c):
                xtc = xin.tile([128, 8, 512], BF16, tag="xt", name=f"xt{c}")
                nc.sync.dma_start(
                    xtc[:],
                    xT[:, 512 * c : 512 * (c + 1)].rearrange(
                        "(o p) s -> p o s", p=128
                    ),
                )
                xts[c] = xtc

            if n_mask_tiles <= 4:
                load_x(0)
            nc.sync.dma_start(wk_sb[:], wk.rearrange("(o p) e -> p o e", p=128))
            nc.sync.dma_start(wv_sb[:], wv.rearrange("(o p) e -> p o e", p=128))
            if n_mask_tiles <= 4:
                for c in range(1, 4):
                    load_x(c)
            if n_mask_tiles:
                # causal: host doubles the mask along q so one DVE multiply
                # covers both heads' [128,1024] P tile; general keeps 512.
                mq = 1024 if mode == "causal" else 512
                mt_sb = const.tile([128, n_mask_tiles, mq], BF16)
                nc.sync.dma_start(mt_sb[:], mt.rearrange("m p q -> p m q"))
            if n_mask_tiles <= 4:
                for c in range(4, NCH):
                    load_x(c)
            wo_sb = const.tile([128, 8, DM], BF16)
            nc.sync.dma_start(wo_sb[:], wo.rearrange("(o p) n -> p o n", p=128))

            qT_sb = const.tile([128, NCH, 512], BF16)
            kT_sb = const.tile([128, NCH, 512], BF16)
            # V in [token, feature] layout, per k-tile, per head:
            # [p=token%128, ktile, head, 80] cols 0:64 = v, col 64 = 1.0
            v_sb = const.tile([128, T // 128, HP, 80], BF16)
            nc.vector.memset(v_sb[:, :, :, 64:65], 1.0)
            identb = const.tile([128, 128], BF16)
            make_identity(nc, identb[:])
            # sel[k, p] = (p // 64 == k): K=2 stationary that broadcasts the
            # per-(head-parity, token) reciprocal to the matching partitions.
            # (host-supplied: sub-partition memsets fail BIR verification)
            magic2 = const.tile([2, 1024], I32)
            nc.sync.dma_start(magic2[:], rmag_t[:, :])
            sel = const.tile([2, 128], BF16)
            nc.sync.dma_start(sel[:], sel_t[:, :])

            warm_in = dram.tile([NCORES, 1, 2], BF16, name="warm_in")
            warm_out = dram.tile([NCORES, 1, 2], BF16, name="warm_out")
            nc.gpsimd.collective_compute(
                "AllToAll",
                mybir.AluOpType.bypass,
                replica_groups=[list(range(NCORES))],
                ins=[warm_in.opt()],
                outs=[warm_out.opt()],
            )
            # one A2A per q-tile, carrying both batches; per-dest payload is
            # [130, 64]: rows 0:128 = unnormalized attnT columns, rows
            # 128:130 = the two heads' softmax denominators for those tokens.
            a2a_in = [
                dramp[qt].tile([NCORES, B, 130, 64], BF16, name=f"a2a_in{qt}")
                for qt in range(QT_PER_S)
            ]
            a2a_out = [
                dramp[qt].tile([NCORES, B, 130, 64], BF16, name=f"a2a_out{qt}")
                for qt in range(QT_PER_S)
            ]

            def qkv_chunk(c):
                if c in xts:
                    xt = xts[c]
                else:
                    xt = xin.tile(
                        [128, 8, 512], BF16, tag="xt", name=f"xt{c}"
                    )
                    nc.sync.dma_start(
                        xt[:],
                        xT[:, 512 * c : 512 * (c + 1)].rearrange(
                            "(o p) s -> p o s", p=128
                        ),
                    )
                for name, w_sb, dst in (
                    ("q", wq_sb, qT_sb),
                    ("k", wk_sb, kT_sb),
                ):
                    psum = ps_misc.tile(
                        [128, 512], F32, tag="psm", name=f"ps_{name}{c}"
                    )
                    for kt in range(8):
                        nc.tensor.matmul(
                            psum[:],
                            w_sb[:, kt, :],
                            xt[:, kt, :],
                            start=(kt == 0),
                            stop=(kt == 7),
                        )
                    nc.vector.tensor_copy(dst[:, c, :], psum[:])
                psum = ps_misc.tile([128, 512], F32, tag="psm", name=f"ps_v{c}")
                for kt in range(8):
                    nc.tensor.matmul(
                        psum[:],
                        wv_sb[:, kt, :],
                        xt[:, kt, :],
                        start=(kt == 0),
                        stop=(kt == 7),
                    )
                vstg = stage.tile([128, 512], BF16, tag="vstg")
                nc.vector.tensor_copy(vstg[:], psum[:])
                ps_t = ps_misc.tile(
                    [128, 512], BF16, tag="psm", name=f"ps_t{c}"
                )
                for sub in range(4):
                    nc.tensor.transpose(
                        ps_t[:, 128 * sub : 128 * (sub + 1)],
                        vstg[:, 128 * sub : 128 * (sub + 1)],
                        identb[:],
                    )
                for sub in range(4):
                    ktile = 4 * c + sub
                    nc.vector.tensor_copy(
                        v_sb[:, ktile, :, 0:64],
                        ps_t[:, 128 * sub : 128 * (sub + 1)].rearrange(
                            "p (h d) -> p h d", h=HP
                        ),
                    )

            def attention(b, qt):
                ch = b * QT_PER_S + qt
                nkt = _nkt(qt, mode)
                av = [
                    ps_av.tile([128, 512], F32, tag="av", name=f"av{ch}_{h}")
                    for h in range(HP)
                ]

                def mask_index(kt):
                    if mode == "causal":
                        off = kt - 4 * qt
                        return off if 0 <= off < 4 else None
                    if mode == "general":
                        return qt * KT_PER_S + kt
                    return None

                def emit_scores(kt):
                    """Scores + exp (+ mask) for one k-tile; returns AV srcs.

                    Both heads share one [128,1024] two-bank PSUM tile (the
                    K=64 matmuls auto-row-tile to PE quadrants (0,0)/(64,0)
                    and can run concurrently) so a single ACT exp covers
                    them."""
                    c, ks = b * QT_PER_S + kt // 4, kt % 4
                    ps = ps_s.tile(
                        [128, 1024], F32, tag="ps_s", name=f"s{ch}_{kt}"
                    )
                    for h in range(HP):
                        nc.tensor.matmul(
                            ps[:, 512 * h : 512 * (h + 1)],
                            kT_sb[
                                64 * h : 64 * (h + 1),
                                c,
                                128 * ks : 128 * (ks + 1),
                            ],
                            qT_sb[64 * h : 64 * (h + 1), ch, :],
                            start=True,
                            stop=True,
                        )
                    pt = pp.tile([128, 1024], BF16, tag="p", bufs=5)
                    nc.scalar.activation(
                        pt[:], ps[:], AF.Exp, scale=float(SCALE)
                    )
                    mi = mask_index(kt)
                    if mi is None:
                        src = pt
                    elif mode == "causal":
                        src = pp.tile([128, 1024], BF16, tag="pm", bufs=4)
                        nc.vector.tensor_tensor(
                            src[:], pt[:], mt_sb[:, mi, :],
                            mybir.AluOpType.mult,
                        )
                    else:
                        src = pp.tile([128, 1024], BF16, tag="pm", bufs=4)
                        for h in range(HP):
                            nc.vector.tensor_tensor(
                                src[:, 512 * h : 512 * (h + 1)],
                                pt[:, 512 * h : 512 * (h + 1)],
                                mt_sb[:, mi, :],
                                mybir.AluOpType.mult,
                            )
                    return src

                def emit_av(kt, src):
                    for h in range(HP):
                        nc.tensor.matmul(
                            av[h][0:65, :],
                            v_sb[:, b * KT_PER_S + kt, h, 0:65],
                            src[:, 512 * h : 512 * (h + 1)],
                            start=(kt == 0),
                            stop=(kt == nkt - 1),
                        )

                # software pipeline: scores run 3 k-tiles ahead of AV so the
                # PE has independent work while ACT/DVE produce P. The score
                # pool (2 two-bank tiles) holds 2 k-tiles in flight; the
                # 3rd's matmul waits only on exp draining tile kt-2.
                pend = []
                for kt in range(nkt):
                    pend.append((kt, emit_scores(kt)))
                    if len(pend) > 3:
                        emit_av(*pend.pop(0))
                while pend:
                    emit_av(*pend.pop(0))
                # epilogue: just cast-copies (on ACT, so the DVE stays free
                # for the next A2A's reciprocal chain) -- normalization
                # happens on the receiving core after the A2A.
                attnT = misc.tile([128, 512], BF16, tag="attnT", bufs=4)
                # denom rows parked at partitions 0 / 64 (engine writes must
                # start at a 0/32/64/96-aligned partition)
                denb = misc.tile([65, 512], BF16, tag="denb", bufs=4)
                for h in range(HP):
                    nc.vector.tensor_copy(
                        attnT[64 * h : 64 * (h + 1), :], av[h][0:64, :]
                    )
                    nc.vector.tensor_copy(
                        denb[64 * h : 64 * h + 1, :], av[h][64:65, :]
                    )
                # payload DMAs split across two queues so the last chunk's
                # A2A isn't gated on one engine draining the descriptors.
                for r in range(NCORES):
                    eng = nc.gpsimd if r % 2 == 0 else nc.sync
                    eng.dma_start(
                        a2a_in[qt][r, b, 0:128, :],
                        attnT[:, 64 * r : 64 * (r + 1)],
                    )
                # NOTE: a single DMA with the partition dim in the middle of
                # the source AP mis-lowers (verified on HW) -- keep 8 simple
                # per-dest transfers.
                for r in range(NCORES):
                    eng2 = nc.sync if r % 2 == 0 else nc.gpsimd
                    eng2.dma_start(
                        a2a_in[qt][r, b, 128:130, :],
                        denb[0:65:64, 64 * r : 64 * (r + 1)],
                    )

            def a2a(qt):
                nc.gpsimd.collective_compute(
                    "AllToAll",
                    mybir.AluOpType.bypass,
                    replica_groups=[list(range(NCORES))],
                    ins=[a2a_in[qt].opt()],
                    outs=[a2a_out[qt].opt()],
                )

            ab_tiles = {}

            def ab_load(qt):
                ab = const.tile([128, 8, 128], BF16, name=f"ab{qt}")
                # ab[p, r, 64b+t] <- a2a_out[qt][r, b, p, t]
                nc.sync.dma_start(
                    ab.rearrange("p r (b t) -> p r b t", b=B),
                    a2a_out[qt][:, :, 0:128, :].rearrange(
                        "r b p t -> p r b t"
                    ),
                )
                dsb = const.tile([2, 8, 2, 64], BF16, name=f"dsb{qt}")
                nc.sync.dma_start(
                    dsb[:],
                    a2a_out[qt][:, :, 128:130, :].rearrange(
                        "r b h t -> h r b t"
                    ),
                )
                # 1/denom via bit-trick seed + one Newton step on DVE (the
                # scheduler slots these into gaps of the mask-mult stream).
                dnf = misc.tile([2, 1024], F32, tag="dnf", bufs=2)
                nc.vector.tensor_copy(
                    dnf[:], dsb.rearrange("h r b t -> h (r b t)")
                )
                nr0 = misc.tile([2, 1024], F32, tag="nr0", bufs=2)
                nc.vector.tensor_tensor(
                    nr0.bitcast(I32), magic2[:], dnf.bitcast(I32),
                    mybir.AluOpType.subtract,
                )
                ne = misc.tile([2, 1024], F32, tag="ne", bufs=2)
                nc.vector.tensor_tensor(
                    ne[:], dnf[:], nr0[:], mybir.AluOpType.mult
                )
                nc.vector.tensor_scalar(
                    ne[:], ne[:], -1.0, 2.0,
                    mybir.AluOpType.mult, mybir.AluOpType.add,
                )
                nrec = misc.tile([2, 1024], BF16, tag="nrec", bufs=2)
                nc.vector.tensor_tensor(
                    nrec[:], nr0[:], ne[:], mybir.AluOpType.mult
                )
                ab_tiles[qt] = (ab, nrec)

            def outproj(qt):
                ab, nrec = ab_tiles[qt]
                # broadcast recip(head_parity, token) to the matching
                # partitions with a K=2 outer product, then scale ab.
                # Emitted here (two attention chunks after the A2A) so the
                # PE-queue Rp matmuls never sit waiting on the collective.
                Rp = ps_s.tile([128, 1024], F32, tag="ps_s", name=f"R{qt}")
                for i in range(2):
                    nc.tensor.matmul(
                        Rp[:, 512 * i : 512 * (i + 1)],
                        sel[:],
                        nrec[:, 512 * i : 512 * (i + 1)],
                        start=True,
                        stop=True,
                    )
                abn = misc.tile([128, 8, 128], BF16, tag="abn", bufs=2)
                nc.vector.tensor_tensor(
                    abn.rearrange("p r t -> p (r t)"),
                    ab.rearrange("p r t -> p (r t)"),
                    Rp[:],
                    mybir.AluOpType.mult,
                )
                ab = abn
                psos = [
                    ps_misc.tile([128, 512], F32, tag="psm", name=f"o{qt}_{h2}")
                    for h2 in range(2)
                ]
                for fb in range(8):
                    for half in range(2):
                        nc.tensor.matmul(
                            psos[half][:],
                            ab[:, fb, :],
                            wo_sb[:, fb, 512 * half : 512 * (half + 1)],
                            start=(fb == 0),
                            stop=(fb == 7),
                        )
                for half in range(2):
                    osb = stage.tile([128, 512], F32, tag="osb", bufs=4)
                    nc.vector.tensor_copy(osb[:], psos[half][:])
                    nc.scalar.dma_start(
                        out[
                            128 * qt : 128 * (qt + 1),
                            512 * half : 512 * (half + 1),
                        ],
                        osb[:],
                    )

            # ---- emission -------------------------------------------------
            def qkv_chunk_s(c):
                with nc.named_scope(f"qkv{c}"):
                    qkv_chunk(c)

            def attention_s(b, qt):
                with nc.named_scope(f"att{b}{qt}"):
                    attention(b, qt)

            def a2a_s(qt):
                with nc.named_scope(f"a2a{qt}"):
                    a2a(qt)

            def ab_load_s(qt):
                with nc.named_scope(f"abld{qt}"):
                    ab_load(qt)

            def outproj_s(qt):
                with nc.named_scope(f"oproj{qt}"):
                    outproj(qt)

            for c in range(4):
                qkv_chunk_s(c)
            attention_s(0, 3)
            for c in range(4, 8):
                qkv_chunk_s(c)
            attention_s(1, 3)
            a2a_s(3)
            attention_s(0, 2)
            attention_s(1, 2)
            a2a_s(2)
            attention_s(0, 1)
            ab_load_s(3)
            attention_s(1, 1)
            a2a_s(1)
            outproj_s(3)
            attention_s(0, 0)
            ab_load_s(2)
            attention_s(1, 0)
            a2a_s(0)
            outproj_s(2)
            ab_load_s(1)
            outproj_s(1)
            ab_load_s(0)
            outproj_s(0)
            _stk.close()
    _split_waits(nc)

    # Encode a hash of the BIR into the shape of an unused dummy input so
    # the HLO (and therefore the NEFF cache key) changes with the kernel.
    import hashlib

    hv = int.from_bytes(
        hashlib.sha256(nc.to_json_bytes()).digest()[:4], "little"
    )
    nonce_shape = [hv % 1021 + 1, (hv // 1021) % 1021 + 1]
    nc.dram_tensor("nonce", nonce_shape, F32, kind="ExternalInput")
    nc._nonce_shape = nonce_shape
    return nc


def _sel_np():
    s = np.zeros((2, 128), ml_dtypes.bfloat16)
    s[0, 0:64] = 1.0
    s[1, 64:128] = 1.0
    return s


_BUILD_CACHE = {}


def _get_nc(mode, n_mask_tiles):
    key = (mode, n_mask_tiles)
    if key not in _BUILD_CACHE:
        _BUILD_CACHE[key] = build(mode, n_mask_tiles)
    return _BUILD_CACHE[key]


def kernel(x, Wqkv, Wo, mask):
    x = np.asarray(x)
    Wqkv = np.asarray(Wqkv)
    Wo = np.asarray(Wo)
    mask = np.asarray(mask)

    m2 = mask.reshape(S, S)
    if np.array_equal(m2, np.tril(np.ones((S, S), bool))):
        mode = "causal"
    elif m2.all():
        mode = "full"
    else:
        mode = "general"

    xT = np.ascontiguousarray(x.reshape(T, DM).T).astype(ml_dtypes.bfloat16)
    w4 = Wqkv.reshape(DM, H, 3, D)

    if mode == "causal":
        qq = np.arange(512)[None, :]
        kk = np.arange(128)[:, None]
        mts = np.stack(
            [(qq - kk >= 128 * o) for o in range(4)]
        ).astype(ml_dtypes.bfloat16)
        mts = np.concatenate([mts, mts], axis=-1)  # both heads' halves
        n_mask_tiles = 4
    elif mode == "general":
        tiles = []
        for qt in range(QT_PER_S):
            for kt in range(KT_PER_S):
                sub = m2[512 * qt : 512 * (qt + 1), 128 * kt : 128 * (kt + 1)]
                tiles.append(sub.T)
        mts = np.stack(tiles).astype(ml_dtypes.bfloat16)
        n_mask_tiles = len(tiles)
    else:
        mts = None
        n_mask_tiles = 0

    nc = _get_nc(mode, n_mask_tiles)

    in_maps = []
    for j in range(NCORES):
        hs = slice(HP * j, HP * (j + 1))
        im = {
            "xT": xT,
            "wq": np.ascontiguousarray(
                w4[:, hs, 0, :].reshape(DM, HP * D)
            ).astype(ml_dtypes.bfloat16),
            "wk": np.ascontiguousarray(
                w4[:, hs, 1, :].reshape(DM, HP * D)
            ).astype(ml_dtypes.bfloat16),
            "wv": np.ascontiguousarray(
                w4[:, hs, 2, :].reshape(DM, HP * D)
            ).astype(ml_dtypes.bfloat16),
            "wo": Wo.astype(ml_dtypes.bfloat16),
            "sel": _sel_np(),
            "rmagic": np.full((2, 1024), RECIP_MAGIC, np.int32),
            "nonce": np.zeros(nc._nonce_shape, np.float32),
        }
        if n_mask_tiles:
            im["mt"] = mts
        in_maps.append(im)

    res = run_bass_kernel_spmd(nc, in_maps, list(range(NCORES)))
    # core j's output rows: [128*qt + 64*b + i] = batch b token
    # 512*qt + 64*j + i.
    full = np.empty((B, S, DM), np.float32)
    for j in range(NCORES):
        o = res.results[j]["out"]
        for qt in range(QT_PER_S):
            for b in range(B):
                full[b, 512 * qt + 64 * j : 512 * qt + 64 * (j + 1), :] = o[
                    128 * qt + 64 * b : 128 * qt + 64 * (b + 1)
                ]
    return full


if __name__ == "__main__":
    rng = np.random.default_rng(0)
    x = rng.standard_normal((B, S, DM), dtype=np.float32)
    Wqkv = rng.standard_normal((DM, 3 * H * D), dtype=np.float32) * DM**-0.5
    Wo = rng.standard_normal((H * D, DM), dtype=np.float32) * (H * D) ** -0.5
    mask = np.tril(np.ones((S, S), bool))[None, None]
    out = kernel(x=x, Wqkv=Wqkv, Wo=Wo, mask=mask)
    print(out.shape, out.dtype)
